# revision 18
# baseline (speedup 1.0000x reference)
"""ComplexPolarAttention Trainium2 kernel.

score_ij = sum_d mag_i,d mag_j,d cos(phase_i,d - phase_j,d)
         = a_i . a_j + b_i . b_j         with a = mag*cos(phase), b = mag*sin(phase)
out_mag   = softmax(score, axis=1) @ mag
out_phase = softmax(score, axis=1) @ phase

Strategy (8 NeuronCores, SPMD, no collectives):
  - Rows (queries) sharded; keys replicated. Per-core inputs are ROTATED
    along the key axis so core c's queries are columns 0..q of its own key
    panel (softmax over keys is permutation invariant), so the query
    operand is a slice of the key panel.
  - Scores are computed transposed, S^T[k_blk=128, q=1024] in PSUM via one
    K=128 f32r matmul pair per key block (exact).  exp is NOT normalized
    (scores < 88 cannot overflow) and is written as BF16 -- bf16's 8-bit
    exponent covers the ~15-octave dynamic range of the attention weights
    (fp8 flushes the off-diagonal mass and fails; measured).
  - exp runs split across TWO engines so it stays off the critical path:
    ACT does most blocks (true exp table); DVE does the rest with a
    one-instruction bf16 Schraudolph (uint16 = s*184.665 + 16248.6, the
    f32->uint16 write rounds to nearest; bit pattern IS bf16 exp(s) within
    +-4%, mean-centered).  The tiny per-weight wobble cancels in the
    softmax normalization (verified: end-to-end rel err ~3e-3).
  - Value matmuls use a PACKED [mag|phase] bf16 stationary (M=128, the full
    PE array) -- halving the baseline's value-matmul column count.  The
    softmax denominator that baseline got from a 65th 'ones' column is
    instead accumulated on the otherwise-idle DVE as a bf16 binary tree
    over the es tiles, finished by one cheap f32r ones-matmul
    (cross-partition sum) at the tail.
  - PSUM: 2x score tiles (4 banks) + 2 value accumulators (2 banks) +
    2 denominator tiles (2 banks) = 8 banks exactly.
"""

import numpy as np
import ml_dtypes
from contextlib import ExitStack

import concourse.bass as bass
import concourse.tile as tile
from concourse import bacc, mybir
from concourse.bass_utils import run_bass_kernel_spmd

F32 = mybir.dt.float32
F32R = mybir.dt.float32r
BF16 = mybir.dt.bfloat16
U16 = mybir.dt.uint16
BF = ml_dtypes.bfloat16

A16 = 184.6649652       # 128 / ln 2
B16 = 16248.6           # 127*128 - 7.4 (mean-centering Schraudolph offset)


def abt_chunk_widths(n):
    widths, rem = [], n
    for w in (512, 512):
        if rem >= w:
            widths.append(w)
            rem -= w
    while rem:
        w = min(1024, rem)
        widths.append(w)
        rem -= w
    return widths


def engine_plan(kblocks, n_dve):
    """'A' (ACT exp) / 'D' (DVE schraudolph) per key block, spread evenly.
    The first 8 blocks (diagonal) and the last 2 (tail latency) stay on
    ACT, whose queue drains in lock-step with the PE."""
    plan = ['A'] * kblocks
    if n_dve <= 0:
        return plan
    lo, hi = 8, kblocks - 2
    stride = (hi - lo) / n_dve
    placed = 0
    for i in range(n_dve):
        pos = lo + int(i * stride + stride / 2)
        if plan[pos] == 'A':
            plan[pos] = 'D'
            placed += 1
    for j in range(hi - 1, lo - 1, -1):
        if placed >= n_dve:
            break
        if plan[j] == 'A':
            plan[j] = 'D'
            placed += 1
    return plan


N_DVE_EXP = 14          # blocks of exp done on DVE (rest on ACT)
VLAG = 3                # value matmuls trail the exp stream by 3 blocks
N_GP_DEN = 16           # denominator accumulate-adds offloaded to GpSimd


def build_program(n=8192, d=64, n_cores=8, enable_asserts=False):
    assert d == 64
    q = n // n_cores
    kblocks = n // 128
    qblk = 512
    nhalf = q // qblk
    assert nhalf == 2 and q == 1024

    nc = bacc.Bacc(
        "TRN2",
        target_bir_lowering=False,
        debug=False,
        enable_asserts=enable_asserts,
        num_devices=n_cores,
    )

    # ---- DRAM I/O (per-core arrays rotated so queries = keys[0:q]) ----
    chunks = abt_chunk_widths(n)
    abt_in = [nc.dram_tensor(f"abt{i}", [128, w], BF16,
                             kind="ExternalInput").ap()
              for i, w in enumerate(chunks)]
    VCH = 8
    vch = kblocks // VCH
    mopv_in = nc.dram_tensor("mopv", [VCH, 128, vch * 128], BF16,
                             kind="ExternalInput").ap()

    om = nc.dram_tensor("om", [128, q], F32, kind="ExternalOutput").ap()
    oden = nc.dram_tensor("oden", [1, q], F32, kind="ExternalOutput").ap()

    ENG = engine_plan(kblocks, N_DVE_EXP)

    with tile.TileContext(nc) as tc, ExitStack() as ctx:
        persist = ctx.enter_context(tc.tile_pool(name="persist", bufs=1))
        epool = ctx.enter_context(tc.tile_pool(name="exps", bufs=8))
        opool = ctx.enter_context(tc.tile_pool(name="outs", bufs=2))
        spool = ctx.enter_context(tc.tile_pool(name="scores", bufs=2,
                                               space="PSUM"))
        vpool = ctx.enter_context(tc.tile_pool(name="vaccum", bufs=1,
                                               space="PSUM"))
        dpool = ctx.enter_context(tc.tile_pool(name="denps", bufs=1,
                                               space="PSUM"))

        abt = persist.tile([128, n], BF16)            # [a|b]^T all keys
        mopv = persist.tile([128, kblocks, 128], BF16)  # [mag|phase] values
        ones32 = persist.tile([128, 1], BF16)
        nc.vector.memset(ones32[:, :], 1.0)

        # key panel chunks on the sync HWDGE queue (chunk 0 gates block 0)
        off = 0
        for i, w in enumerate(chunks):
            nc.sync.dma_start(out=abt[:, off:off + w], in_=abt_in[i])
            off += w
        abq = abt[:, 0:q]

        # value stationaries on the gpsimd SWDGE queue (idle otherwise)
        for vi in range(VCH):
            b0 = vi * vch
            nc.gpsimd.dma_start(out=mopv[:, b0:b0 + vch, :],
                                in_=mopv_in[vi, :, :])

        # PSUM accumulators
        psV = [vpool.tile([128, qblk], F32, name=f"psV{j}", tag=f"psV{j}")
               for j in range(2)]
        psD = [dpool.tile([1, qblk], F32, name=f"psD{j}", tag=f"psD{j}")
               for j in range(2)]

        # Brief PE warm-up (HAM): measured traces show this device enters
        # the kernel already at full clock, so 2 matmuls are insurance for
        # a cold start only -- more just delays the first real matmul,
        # which is gated by the query-panel DMA (~10us) anyway.
        wsrc = persist.tile([128, 512], F32)
        nc.vector.memset(wsrc[:, :], 0.0)
        warm = spool.tile([128, q], F32, name="warm", tag="ss")
        for _ in range(2):      # fp32 dummies: 4 cyc/row
            nc.tensor.matmul(out=warm[0:16, 0:512], lhsT=wsrc[:, 0:16],
                             rhs=wsrc[:, 0:512], start=True, stop=True)

        # denominator: running per-partition accumulators, one chain on the
        # DVE and one on the otherwise-idle GpSimd (SBUF-only engine).
        acc_D = persist.tile([128, q], F32, name="accD")
        acc_G = persist.tile([128, q], F32, name="accG")
        nc.vector.memset(acc_D[:, :], 0.0)
        nc.gpsimd.memset(acc_G[:, :], 0.0)

        def den_add(es, kb):
            eng = nc.gpsimd if (kb % 4 == 1 and kb // 4 < N_GP_DEN) else \
                nc.vector
            acc = acc_G if eng is nc.gpsimd else acc_D
            eng.tensor_tensor(out=acc[:, :], in0=acc[:, :], in1=es[:, :],
                              op=mybir.AluOpType.add)

        def value_mms(es, kb, first, last):
            for j in range(2):
                nc.tensor.matmul(
                    out=psV[j][:, :],
                    lhsT=mopv[:, kb, :],
                    rhs=es[:, j * qblk:(j + 1) * qblk],
                    start=first, stop=last,
                )

        es_hist = []
        for kb in range(kblocks):
            ss = spool.tile([128, q], F32)
            for j in range(2):
                nc.tensor.matmul(
                    out=ss[:, j * qblk:(j + 1) * qblk],
                    lhsT=abt[:, kb * 128:(kb + 1) * 128],
                    rhs=abq[:, j * qblk:(j + 1) * qblk],
                    start=True, stop=True,
                )
            es = epool.tile([128, q], BF16)
            if ENG[kb] == 'A':
                nc.scalar.activation(
                    es[:, :], ss[:, :], mybir.ActivationFunctionType.Exp,
                )
            else:
                nc.vector.tensor_scalar(
                    out=es[:, :].bitcast(U16), in0=ss[:, :],
                    scalar1=A16, scalar2=B16,
                    op0=mybir.AluOpType.mult, op1=mybir.AluOpType.add,
                )
            es_hist.append(es)
            den_add(es, kb)
            if len(es_hist) > VLAG:
                value_mms(es_hist[-1 - VLAG], kb - VLAG,
                          first=(kb == VLAG), last=False)
        for i in range(VLAG, 0, -1):
            value_mms(es_hist[-i], kblocks - i,
                      first=False, last=(i == 1))

        # denominator: merge the two chains, then cross-partition sum via a
        # bf16 ones-matmul (PSUM accumulates in f32)
        denacc = persist.tile([128, q], BF16, name="denacc")
        nc.vector.tensor_tensor(out=denacc[:, :], in0=acc_D[:, :],
                                in1=acc_G[:, :], op=mybir.AluOpType.add)
        for j in range(2):
            nc.tensor.matmul(
                out=psD[j][:, :], lhsT=ones32[:, :],
                rhs=denacc[:, j * qblk:(j + 1) * qblk],
                start=True, stop=True,
            )

        # outputs: PSUM -> SBUF (DVE) -> DRAM
        oV = opool.tile([128, q], F32, tag="oV")
        oD = opool.tile([1, q], F32, tag="oD")
        for j in range(2):
            qsl = slice(j * qblk, (j + 1) * qblk)
            nc.vector.tensor_copy(oV[:, qsl], psV[j][:, :])
            nc.vector.tensor_copy(oD[:, qsl], psD[j][:, :])
        nc.sync.dma_start(out=om, in_=oV[:, :])
        nc.sync.dma_start(out=oden, in_=oD[:, :])

    nc.compile()
    return nc


def make_inputs(mag, phase, n_cores=8):
    """Host-side prep -> per-core (key-rotated) input maps."""
    n, d = mag.shape
    q = n // n_cores
    kblocks = n // 128
    mag = np.ascontiguousarray(mag, dtype=np.float32)
    phase = np.ascontiguousarray(phase, dtype=np.float32)

    a = mag * np.cos(phase)
    b = mag * np.sin(phase)
    abt_g = np.concatenate([a.T, b.T], axis=0).astype(BF)          # [128, n]
    V = np.concatenate([mag, phase], axis=1).astype(BF)            # [n, 128]

    chunks = abt_chunk_widths(n)
    VCH = 8
    vch = kblocks // VCH

    in_maps = []
    for c in range(n_cores):
        r = c * q
        abt_c = np.roll(abt_g, -r, axis=1)
        Vr = np.roll(V, -r, axis=0)
        # mopv[vi][p][blk*128 + f] = V[(vi*vch+blk)*128 + p, f]
        mo = np.ascontiguousarray(
            Vr.reshape(VCH, vch, 128, 128).transpose(0, 2, 1, 3)
            .reshape(VCH, 128, vch * 128))
        m = {"mopv": mo}
        off = 0
        for i, w in enumerate(chunks):
            m[f"abt{i}"] = np.ascontiguousarray(abt_c[:, off:off + w])
            off += w
        in_maps.append(m)
    return in_maps


def gather_outputs(results, n, d, n_cores=8):
    new_mag = np.empty((n, d), np.float32)
    new_phase = np.empty((n, d), np.float32)
    q = n // n_cores
    for c in range(n_cores):
        om = results[c]["om"]            # [128, q]: rows 0:64 mag, 64:128 ph
        den = results[c]["oden"][0]      # [q]
        qsl = slice(c * q, (c + 1) * q)
        new_mag[qsl] = (om[0:64, :] / den).T
        new_phase[qsl] = (om[64:128, :] / den).T
    return new_mag, new_phase


_PROGRAM_CACHE = {}


def _get_program(n, d, n_cores):
    key = (n, d, n_cores)
    if key not in _PROGRAM_CACHE:
        _PROGRAM_CACHE[key] = build_program(n=n, d=d, n_cores=n_cores)
    return _PROGRAM_CACHE[key]


def kernel(mag, phase):
    mag = np.asarray(mag, dtype=np.float32)
    phase = np.asarray(phase, dtype=np.float32)
    n, d = mag.shape
    n_cores = 8
    nc = _get_program(n, d, n_cores)
    in_maps = make_inputs(mag, phase, n_cores=n_cores)
    res = run_bass_kernel_spmd(nc, in_maps, list(range(n_cores)))
    return gather_outputs(res.results, n, d, n_cores=n_cores)


# revision 25
# speedup vs baseline: 1.3626x; 1.3626x over previous
"""ComplexPolarAttention Trainium2 kernel.

score_ij = sum_d mag_i,d mag_j,d cos(phase_i,d - phase_j,d)
         = a_i . a_j + b_i . b_j         with a = mag*cos(phase), b = mag*sin(phase)
out_mag   = softmax(score, axis=1) @ mag
out_phase = softmax(score, axis=1) @ phase

Strategy (8 NeuronCores, SPMD, no collectives):
  - Rows (queries) sharded; keys replicated. Per-core inputs are ROTATED
    along the key axis so core c's queries are columns 0..q of its own key
    panel (softmax over keys is permutation invariant), so the query
    operand is a slice of the key panel.
  - Scores are computed transposed, S^T[k_blk=128, q=1024] in PSUM via one
    K=128 f32r matmul pair per key block (exact).  exp is NOT normalized
    (scores < 88 cannot overflow) and is written as BF16 -- bf16's 8-bit
    exponent covers the ~15-octave dynamic range of the attention weights
    (fp8 flushes the off-diagonal mass and fails; measured).
  - exp runs split across TWO engines so it stays off the critical path:
    ACT does most blocks (true exp table); DVE does the rest with a
    one-instruction bf16 Schraudolph (uint16 = s*184.665 + 16248.6, the
    f32->uint16 write rounds to nearest; bit pattern IS bf16 exp(s) within
    +-4%, mean-centered).  The tiny per-weight wobble cancels in the
    softmax normalization (verified: end-to-end rel err ~3e-3).
  - Value matmuls use a PACKED [mag|phase] bf16 stationary (M=128, the full
    PE array) -- halving the baseline's value-matmul column count.  The
    softmax denominator that baseline got from a 65th 'ones' column is
    instead accumulated on the otherwise-idle DVE as a bf16 binary tree
    over the es tiles, finished by one cheap f32r ones-matmul
    (cross-partition sum) at the tail.
  - PSUM: 2x score tiles (4 banks) + 2 value accumulators (2 banks) +
    2 denominator tiles (2 banks) = 8 banks exactly.
"""

import numpy as np
import ml_dtypes
from contextlib import ExitStack

import concourse.bass as bass
import concourse.tile as tile
from concourse import bacc, mybir
from concourse.bass_utils import run_bass_kernel_spmd

F32 = mybir.dt.float32
F32R = mybir.dt.float32r
BF16 = mybir.dt.bfloat16
U16 = mybir.dt.uint16
BF = ml_dtypes.bfloat16

A16 = 184.6649652       # 128 / ln 2
B16 = 16248.6           # 127*128 - 7.4 (mean-centering Schraudolph offset)


def abt_chunk_widths(n):
    """First chunk covers the whole query panel (cols 0..1023) so one DMA
    gates the first score matmul; the rest stream in 1024-wide."""
    widths, rem = [], n
    while rem:
        w = min(1024, rem)
        widths.append(w)
        rem -= w
    return widths


def engine_plan(kblocks, n_dve):
    """'A' (ACT exp) / 'D' (DVE schraudolph) per key block, spread evenly.
    The first 8 blocks (diagonal) and the last 2 (tail latency) stay on
    ACT, whose queue drains in lock-step with the PE."""
    plan = ['A'] * kblocks
    if n_dve <= 0:
        return plan
    lo, hi = 8, kblocks - 2
    stride = (hi - lo) / n_dve
    placed = 0
    for i in range(n_dve):
        pos = lo + int(i * stride + stride / 2)
        if plan[pos] == 'A':
            plan[pos] = 'D'
            placed += 1
    for j in range(hi - 1, lo - 1, -1):
        if placed >= n_dve:
            break
        if plan[j] == 'A':
            plan[j] = 'D'
            placed += 1
    return plan


N_DVE_EXP = 14          # blocks of exp done on DVE (rest on ACT)
VLAG = 3                # value matmuls trail the exp stream by 3 blocks
GP_L1_PAIRS = {2, 6, 10, 14, 18, 22, 26, 30}   # den L1 adds on GpSimd


def build_program(n=8192, d=64, n_cores=8, enable_asserts=False):
    assert d == 64
    q = n // n_cores
    kblocks = n // 128
    qblk = 512
    nhalf = q // qblk
    assert nhalf == 2 and q == 1024

    nc = bacc.Bacc(
        "TRN2",
        target_bir_lowering=False,
        debug=False,
        enable_asserts=enable_asserts,
        num_devices=n_cores,
    )

    # ---- DRAM I/O (per-core arrays rotated so queries = keys[0:q]) ----
    chunks = abt_chunk_widths(n)
    abt_in = [nc.dram_tensor(f"abt{i}", [128, w], BF16,
                             kind="ExternalInput").ap()
              for i, w in enumerate(chunks)]
    VCH = 8
    vch = kblocks // VCH
    mopv_in = nc.dram_tensor("mopv", [VCH, 128, vch * 128], BF16,
                             kind="ExternalInput").ap()

    om = nc.dram_tensor("om", [128, q], F32, kind="ExternalOutput").ap()
    oden = nc.dram_tensor("oden", [1, q], F32, kind="ExternalOutput").ap()

    ENG = engine_plan(kblocks, N_DVE_EXP)

    with tile.TileContext(nc) as tc, ExitStack() as ctx:
        persist = ctx.enter_context(tc.tile_pool(name="persist", bufs=1))
        epool = ctx.enter_context(tc.tile_pool(name="exps", bufs=8))
        tpool = [ctx.enter_context(tc.tile_pool(name=f"tree{i}", bufs=2))
                 for i in range(5)]
        opool = ctx.enter_context(tc.tile_pool(name="outs", bufs=2))
        spool = ctx.enter_context(tc.tile_pool(name="scores", bufs=2,
                                               space="PSUM"))
        vpool = ctx.enter_context(tc.tile_pool(name="vaccum", bufs=1,
                                               space="PSUM"))
        dpool = ctx.enter_context(tc.tile_pool(name="denps", bufs=1,
                                               space="PSUM"))

        abt = persist.tile([128, n], BF16)            # [a|b]^T all keys
        mopv = persist.tile([128, kblocks, 128], BF16)  # [mag|phase] values
        ones32 = persist.tile([128, 1], BF16)
        nc.vector.memset(ones32[:, :], 1.0)

        # key panel chunks on the sync HWDGE queue (chunk 0 gates block 0)
        off = 0
        for i, w in enumerate(chunks):
            nc.sync.dma_start(out=abt[:, off:off + w], in_=abt_in[i])
            off += w
        abq = abt[:, 0:q]

        # value stationaries on the gpsimd SWDGE queue (idle otherwise)
        for vi in range(VCH):
            b0 = vi * vch
            nc.gpsimd.dma_start(out=mopv[:, b0:b0 + vch, :],
                                in_=mopv_in[vi, :, :])

        # PSUM accumulators
        psV = [vpool.tile([128, qblk], F32, name=f"psV{j}", tag=f"psV{j}")
               for j in range(2)]
        psD = [dpool.tile([1, qblk], F32, name=f"psD{j}", tag=f"psD{j}")
               for j in range(2)]

        # Brief PE warm-up (HAM): measured traces show this device enters
        # the kernel already at full clock, so 2 matmuls are insurance for
        # a cold start only -- more just delays the first real matmul,
        # which is gated by the query-panel DMA (~10us) anyway.
        wsrc = persist.tile([128, 512], F32)
        nc.vector.memset(wsrc[:, :], 0.0)
        warm = spool.tile([128, q], F32, name="warm", tag="ss")
        for _ in range(2):      # fp32 dummies: 4 cyc/row
            nc.tensor.matmul(out=warm[0:16, 0:512], lhsT=wsrc[:, 0:16],
                             rhs=wsrc[:, 0:512], start=True, stop=True)

        # Denominator binary tree, all-bf16 (f32/in-place DVE adds run at
        # 1x and serialize -- measured 1602ns vs 653ns for bf16 ones).
        # L1 tiles (block pairs) enter at level 1; 32 -> 16 -> 8 -> 4 ->
        # 2 -> 1.  A quarter of the L1 adds run on the otherwise-idle
        # GpSimd (~2.8us each there, but off the DVE critical engine).
        pend = [None] * 7
        acc_holder = [None]

        def tree_feed(level, t):
            if level == 6:
                acc_holder[0] = t
                return
            if pend[level] is None:
                pend[level] = t
                return
            s = pend[level]
            pend[level] = None
            if level < 5:
                o = tpool[min(level, 4)].tile([128, q], BF16,
                                              name=f"t{level}")
            else:
                o = persist.tile([128, q], BF16, name="denacc")
            nc.vector.tensor_tensor(out=o[:, :], in0=s[:, :], in1=t[:, :],
                                    op=mybir.AluOpType.add)
            tree_feed(level + 1, o)

        def value_mms(es, kb, first, last):
            for j in range(2):
                nc.tensor.matmul(
                    out=psV[j][:, :],
                    lhsT=mopv[:, kb, :],
                    rhs=es[:, j * qblk:(j + 1) * qblk],
                    start=first, stop=last,
                )

        es_hist = []
        for kb in range(kblocks):
            ss = spool.tile([128, q], F32)
            for j in range(2):
                nc.tensor.matmul(
                    out=ss[:, j * qblk:(j + 1) * qblk],
                    lhsT=abt[:, kb * 128:(kb + 1) * 128],
                    rhs=abq[:, j * qblk:(j + 1) * qblk],
                    start=True, stop=True,
                )
            es = epool.tile([128, q], BF16)
            if ENG[kb] == 'A':
                nc.scalar.activation(
                    es[:, :], ss[:, :], mybir.ActivationFunctionType.Exp,
                )
            else:
                nc.vector.tensor_scalar(
                    out=es[:, :].bitcast(U16), in0=ss[:, :],
                    scalar1=A16, scalar2=B16,
                    op0=mybir.AluOpType.mult, op1=mybir.AluOpType.add,
                )
            es_hist.append(es)
            # den tree leaf: pair the two newest es tiles
            if kb % 2 == 1:
                pair = kb // 2
                eng = nc.gpsimd if pair in GP_L1_PAIRS else nc.vector
                l1 = tpool[0].tile([128, q], BF16)
                eng.tensor_tensor(out=l1[:, :], in0=es_hist[-2][:, :],
                                  in1=es_hist[-1][:, :],
                                  op=mybir.AluOpType.add)
                tree_feed(1, l1)
            if len(es_hist) > VLAG:
                value_mms(es_hist[-1 - VLAG], kb - VLAG,
                          first=(kb == VLAG), last=False)
        for i in range(VLAG, 0, -1):
            value_mms(es_hist[-i], kblocks - i,
                      first=False, last=(i == 1))

        # denominator: cross-partition sum via a bf16 ones-matmul (PSUM
        # accumulates in f32)
        denacc = acc_holder[0]
        assert denacc is not None
        for j in range(2):
            nc.tensor.matmul(
                out=psD[j][:, :], lhsT=ones32[:, :],
                rhs=denacc[:, j * qblk:(j + 1) * qblk],
                start=True, stop=True,
            )

        # outputs: PSUM -> SBUF (DVE) -> DRAM
        oV = opool.tile([128, q], F32, tag="oV")
        oD = opool.tile([1, q], F32, tag="oD")
        for j in range(2):
            qsl = slice(j * qblk, (j + 1) * qblk)
            nc.vector.tensor_copy(oV[:, qsl], psV[j][:, :])
            nc.vector.tensor_copy(oD[:, qsl], psD[j][:, :])
        nc.sync.dma_start(out=om, in_=oV[:, :])
        nc.sync.dma_start(out=oden, in_=oD[:, :])

    nc.compile()
    return nc


def make_inputs(mag, phase, n_cores=8):
    """Host-side prep -> per-core (key-rotated) input maps."""
    n, d = mag.shape
    q = n // n_cores
    kblocks = n // 128
    mag = np.ascontiguousarray(mag, dtype=np.float32)
    phase = np.ascontiguousarray(phase, dtype=np.float32)

    a = mag * np.cos(phase)
    b = mag * np.sin(phase)
    abt_g = np.concatenate([a.T, b.T], axis=0).astype(BF)          # [128, n]
    V = np.concatenate([mag, phase], axis=1).astype(BF)            # [n, 128]

    chunks = abt_chunk_widths(n)
    VCH = 8
    vch = kblocks // VCH

    in_maps = []
    for c in range(n_cores):
        r = c * q
        abt_c = np.roll(abt_g, -r, axis=1)
        Vr = np.roll(V, -r, axis=0)
        # mopv[vi][p][blk*128 + f] = V[(vi*vch+blk)*128 + p, f]
        mo = np.ascontiguousarray(
            Vr.reshape(VCH, vch, 128, 128).transpose(0, 2, 1, 3)
            .reshape(VCH, 128, vch * 128))
        m = {"mopv": mo}
        off = 0
        for i, w in enumerate(chunks):
            m[f"abt{i}"] = np.ascontiguousarray(abt_c[:, off:off + w])
            off += w
        in_maps.append(m)
    return in_maps


def gather_outputs(results, n, d, n_cores=8):
    new_mag = np.empty((n, d), np.float32)
    new_phase = np.empty((n, d), np.float32)
    q = n // n_cores
    for c in range(n_cores):
        om = results[c]["om"]            # [128, q]: rows 0:64 mag, 64:128 ph
        den = results[c]["oden"][0]      # [q]
        qsl = slice(c * q, (c + 1) * q)
        new_mag[qsl] = (om[0:64, :] / den).T
        new_phase[qsl] = (om[64:128, :] / den).T
    return new_mag, new_phase


_PROGRAM_CACHE = {}


def _get_program(n, d, n_cores):
    key = (n, d, n_cores)
    if key not in _PROGRAM_CACHE:
        _PROGRAM_CACHE[key] = build_program(n=n, d=d, n_cores=n_cores)
    return _PROGRAM_CACHE[key]


def kernel(mag, phase):
    mag = np.asarray(mag, dtype=np.float32)
    phase = np.asarray(phase, dtype=np.float32)
    n, d = mag.shape
    n_cores = 8
    nc = _get_program(n, d, n_cores)
    in_maps = make_inputs(mag, phase, n_cores=n_cores)
    res = run_bass_kernel_spmd(nc, in_maps, list(range(n_cores)))
    return gather_outputs(res.results, n, d, n_cores=n_cores)


# revision 28
# speedup vs baseline: 1.4079x; 1.0332x over previous
"""ComplexPolarAttention Trainium2 kernel.

score_ij = sum_d mag_i,d mag_j,d cos(phase_i,d - phase_j,d)
         = a_i . a_j + b_i . b_j         with a = mag*cos(phase), b = mag*sin(phase)
out_mag   = softmax(score, axis=1) @ mag
out_phase = softmax(score, axis=1) @ phase

Strategy (8 NeuronCores, SPMD, no collectives):
  - Rows (queries) sharded; keys replicated. Per-core inputs are ROTATED
    along the key axis so core c's queries are columns 0..q of its own key
    panel (softmax over keys is permutation invariant), so the query
    operand is a slice of the key panel.
  - Scores are computed transposed, S^T[k_blk=128, q=1024] in PSUM via one
    K=128 f32r matmul pair per key block (exact).  exp is NOT normalized
    (scores < 88 cannot overflow) and is written as BF16 -- bf16's 8-bit
    exponent covers the ~15-octave dynamic range of the attention weights
    (fp8 flushes the off-diagonal mass and fails; measured).
  - exp runs split across TWO engines so it stays off the critical path:
    ACT does most blocks (true exp table); DVE does the rest with a
    one-instruction bf16 Schraudolph (uint16 = s*184.665 + 16248.6, the
    f32->uint16 write rounds to nearest; bit pattern IS bf16 exp(s) within
    +-4%, mean-centered).  The tiny per-weight wobble cancels in the
    softmax normalization (verified: end-to-end rel err ~3e-3).
  - Value matmuls use a PACKED [mag|phase] bf16 stationary (M=128, the full
    PE array) -- halving the baseline's value-matmul column count.  The
    softmax denominator that baseline got from a 65th 'ones' column is
    instead accumulated on the otherwise-idle DVE as a bf16 binary tree
    over the es tiles, finished by one cheap f32r ones-matmul
    (cross-partition sum) at the tail.
  - PSUM: 2x score tiles (4 banks) + 2 value accumulators (2 banks) +
    2 denominator tiles (2 banks) = 8 banks exactly.
"""

import numpy as np
import ml_dtypes
from contextlib import ExitStack

import concourse.bass as bass
import concourse.tile as tile
from concourse import bacc, mybir
from concourse.bass_utils import run_bass_kernel_spmd

F32 = mybir.dt.float32
F32R = mybir.dt.float32r
BF16 = mybir.dt.bfloat16
U16 = mybir.dt.uint16
BF = ml_dtypes.bfloat16

A16 = 184.6649652       # 128 / ln 2
B16 = 16248.6           # 127*128 - 7.4 (mean-centering Schraudolph offset)


def abt_chunk_widths(n):
    """First chunk covers the whole query panel (cols 0..1023) so one DMA
    gates the first score matmul; the rest stream in 1024-wide."""
    widths, rem = [], n
    while rem:
        w = min(1024, rem)
        widths.append(w)
        rem -= w
    return widths


def engine_plan(kblocks, n_dve):
    """'A' (ACT exp) / 'D' (DVE schraudolph) per key block, spread evenly.
    The first 8 blocks (diagonal) and the last 2 (tail latency) stay on
    ACT, whose queue drains in lock-step with the PE."""
    plan = ['A'] * kblocks
    if n_dve <= 0:
        return plan
    lo, hi = 8, kblocks - 6
    stride = (hi - lo) / n_dve
    placed = 0
    for i in range(n_dve):
        pos = lo + int(i * stride + stride / 2)
        if plan[pos] == 'A':
            plan[pos] = 'D'
            placed += 1
    for j in range(hi - 1, lo - 1, -1):
        if placed >= n_dve:
            break
        if plan[j] == 'A':
            plan[j] = 'D'
            placed += 1
    return plan


N_DVE_EXP = 14          # blocks of exp done on DVE (rest on ACT)
VLAG = 3                # value matmuls trail the exp stream by 3 blocks
GP_L1_PAIRS = {2, 6, 10, 14, 18, 22, 26}       # den L1 adds on GpSimd


def build_program(n=8192, d=64, n_cores=8, enable_asserts=False):
    assert d == 64
    q = n // n_cores
    kblocks = n // 128
    qblk = 512
    nhalf = q // qblk
    assert nhalf == 2 and q == 1024

    nc = bacc.Bacc(
        "TRN2",
        target_bir_lowering=False,
        debug=False,
        enable_asserts=enable_asserts,
        num_devices=n_cores,
    )

    # ---- DRAM I/O (per-core arrays rotated so queries = keys[0:q]) ----
    chunks = abt_chunk_widths(n)
    abt_in = [nc.dram_tensor(f"abt{i}", [128, w], BF16,
                             kind="ExternalInput").ap()
              for i, w in enumerate(chunks)]
    VCH = 8
    vch = kblocks // VCH
    mopv_in = nc.dram_tensor("mopv", [VCH, 128, vch * 128], BF16,
                             kind="ExternalInput").ap()

    om = nc.dram_tensor("om", [128, q], F32, kind="ExternalOutput").ap()
    oden = nc.dram_tensor("oden", [1, q], F32, kind="ExternalOutput").ap()

    ENG = engine_plan(kblocks, N_DVE_EXP)

    with tile.TileContext(nc) as tc, ExitStack() as ctx:
        persist = ctx.enter_context(tc.tile_pool(name="persist", bufs=1))
        epool = ctx.enter_context(tc.tile_pool(name="exps", bufs=10))
        tpool = [ctx.enter_context(tc.tile_pool(name=f"tree{i}", bufs=2))
                 for i in range(5)]
        opool = ctx.enter_context(tc.tile_pool(name="outs", bufs=2))
        spool = ctx.enter_context(tc.tile_pool(name="scores", bufs=2,
                                               space="PSUM"))
        vpool = ctx.enter_context(tc.tile_pool(name="vaccum", bufs=1,
                                               space="PSUM"))
        dpool = ctx.enter_context(tc.tile_pool(name="denps", bufs=1,
                                               space="PSUM"))

        abt = persist.tile([128, n], BF16)            # [a|b]^T all keys
        mopv = persist.tile([128, kblocks, 128], BF16)  # [mag|phase] values
        ones32 = persist.tile([128, 1], BF16)
        nc.vector.memset(ones32[:, :], 1.0)

        # key panel chunks on the sync HWDGE queue (chunk 0 gates block 0)
        off = 0
        for i, w in enumerate(chunks):
            nc.sync.dma_start(out=abt[:, off:off + w], in_=abt_in[i])
            off += w
        abq = abt[:, 0:q]

        # value stationaries on the gpsimd SWDGE queue (idle otherwise)
        for vi in range(VCH):
            b0 = vi * vch
            nc.gpsimd.dma_start(out=mopv[:, b0:b0 + vch, :],
                                in_=mopv_in[vi, :, :])

        # PSUM accumulators
        psV = [vpool.tile([128, qblk], F32, name=f"psV{j}", tag=f"psV{j}")
               for j in range(2)]
        psD = [dpool.tile([1, qblk], F32, name=f"psD{j}", tag=f"psD{j}")
               for j in range(2)]

        # Brief PE warm-up (HAM): measured traces show this device enters
        # the kernel already at full clock, so 2 matmuls are insurance for
        # a cold start only -- more just delays the first real matmul,
        # which is gated by the query-panel DMA (~10us) anyway.
        wsrc = persist.tile([128, 512], F32)
        nc.vector.memset(wsrc[:, :], 0.0)
        warm = spool.tile([128, q], F32, name="warm", tag="ss")
        for _ in range(2):      # fp32 dummies: 4 cyc/row
            nc.tensor.matmul(out=warm[0:16, 0:512], lhsT=wsrc[:, 0:16],
                             rhs=wsrc[:, 0:512], start=True, stop=True)

        # Denominator binary tree, all-bf16 (f32/in-place DVE adds run at
        # 1x and serialize -- measured 1602ns vs 653ns for bf16 ones).
        # L1 tiles (block pairs) enter at level 1; 32 -> 16 -> 8 -> 4 ->
        # 2 -> 1.  A quarter of the L1 adds run on the otherwise-idle
        # GpSimd (~2.8us each there, but off the DVE critical engine).
        pend = [None] * 7
        acc_holder = [None]

        def tree_feed(level, t):
            if level == 6:
                acc_holder[0] = t
                return
            if pend[level] is None:
                pend[level] = t
                return
            s = pend[level]
            pend[level] = None
            if level < 5:
                o = tpool[min(level, 4)].tile([128, q], BF16,
                                              name=f"t{level}")
            else:
                o = persist.tile([128, q], BF16, name="denacc")
            nc.vector.tensor_tensor(out=o[:, :], in0=s[:, :], in1=t[:, :],
                                    op=mybir.AluOpType.add)
            tree_feed(level + 1, o)

        def value_mms(es, kb, first, last):
            for j in range(2):
                nc.tensor.matmul(
                    out=psV[j][:, :],
                    lhsT=mopv[:, kb, :],
                    rhs=es[:, j * qblk:(j + 1) * qblk],
                    start=first, stop=last,
                )

        es_hist = []
        for kb in range(kblocks):
            ss = spool.tile([128, q], F32)
            for j in range(2):
                nc.tensor.matmul(
                    out=ss[:, j * qblk:(j + 1) * qblk],
                    lhsT=abt[:, kb * 128:(kb + 1) * 128],
                    rhs=abq[:, j * qblk:(j + 1) * qblk],
                    start=True, stop=True,
                )
            es = epool.tile([128, q], BF16)
            if ENG[kb] == 'A':
                nc.scalar.activation(
                    es[:, :], ss[:, :], mybir.ActivationFunctionType.Exp,
                )
            else:
                nc.vector.tensor_scalar(
                    out=es[:, :].bitcast(U16), in0=ss[:, :],
                    scalar1=A16, scalar2=B16,
                    op0=mybir.AluOpType.mult, op1=mybir.AluOpType.add,
                )
            es_hist.append(es)
            # den tree leaf: pair the two newest es tiles
            if kb % 2 == 1:
                pair = kb // 2
                eng = nc.gpsimd if pair in GP_L1_PAIRS else nc.vector
                l1 = tpool[0].tile([128, q], BF16)
                eng.tensor_tensor(out=l1[:, :], in0=es_hist[-2][:, :],
                                  in1=es_hist[-1][:, :],
                                  op=mybir.AluOpType.add)
                tree_feed(1, l1)
            if len(es_hist) > VLAG:
                value_mms(es_hist[-1 - VLAG], kb - VLAG,
                          first=(kb == VLAG), last=False)
        for i in range(VLAG, 0, -1):
            value_mms(es_hist[-i], kblocks - i,
                      first=False, last=(i == 1))

        # denominator: cross-partition sum via a bf16 ones-matmul (PSUM
        # accumulates in f32)
        denacc = acc_holder[0]
        assert denacc is not None
        for j in range(2):
            nc.tensor.matmul(
                out=psD[j][:, :], lhsT=ones32[:, :],
                rhs=denacc[:, j * qblk:(j + 1) * qblk],
                start=True, stop=True,
            )

        # outputs: PSUM -> SBUF (DVE) -> DRAM
        oV = opool.tile([128, q], F32, tag="oV")
        oD = opool.tile([1, q], F32, tag="oD")
        for j in range(2):
            qsl = slice(j * qblk, (j + 1) * qblk)
            nc.vector.tensor_copy(oV[:, qsl], psV[j][:, :])
            nc.vector.tensor_copy(oD[:, qsl], psD[j][:, :])
        nc.sync.dma_start(out=om, in_=oV[:, :])
        nc.sync.dma_start(out=oden, in_=oD[:, :])

    nc.compile()
    return nc


def make_inputs(mag, phase, n_cores=8):
    """Host-side prep -> per-core (key-rotated) input maps."""
    n, d = mag.shape
    q = n // n_cores
    kblocks = n // 128
    mag = np.ascontiguousarray(mag, dtype=np.float32)
    phase = np.ascontiguousarray(phase, dtype=np.float32)

    a = mag * np.cos(phase)
    b = mag * np.sin(phase)
    abt_g = np.concatenate([a.T, b.T], axis=0).astype(BF)          # [128, n]
    V = np.concatenate([mag, phase], axis=1).astype(BF)            # [n, 128]

    chunks = abt_chunk_widths(n)
    VCH = 8
    vch = kblocks // VCH

    in_maps = []
    for c in range(n_cores):
        r = c * q
        abt_c = np.roll(abt_g, -r, axis=1)
        Vr = np.roll(V, -r, axis=0)
        # mopv[vi][p][blk*128 + f] = V[(vi*vch+blk)*128 + p, f]
        mo = np.ascontiguousarray(
            Vr.reshape(VCH, vch, 128, 128).transpose(0, 2, 1, 3)
            .reshape(VCH, 128, vch * 128))
        m = {"mopv": mo}
        off = 0
        for i, w in enumerate(chunks):
            m[f"abt{i}"] = np.ascontiguousarray(abt_c[:, off:off + w])
            off += w
        in_maps.append(m)
    return in_maps


def gather_outputs(results, n, d, n_cores=8):
    new_mag = np.empty((n, d), np.float32)
    new_phase = np.empty((n, d), np.float32)
    q = n // n_cores
    for c in range(n_cores):
        om = results[c]["om"]            # [128, q]: rows 0:64 mag, 64:128 ph
        den = results[c]["oden"][0]      # [q]
        qsl = slice(c * q, (c + 1) * q)
        new_mag[qsl] = (om[0:64, :] / den).T
        new_phase[qsl] = (om[64:128, :] / den).T
    return new_mag, new_phase


_PROGRAM_CACHE = {}


def _get_program(n, d, n_cores):
    key = (n, d, n_cores)
    if key not in _PROGRAM_CACHE:
        _PROGRAM_CACHE[key] = build_program(n=n, d=d, n_cores=n_cores)
    return _PROGRAM_CACHE[key]


def kernel(mag, phase):
    mag = np.asarray(mag, dtype=np.float32)
    phase = np.asarray(phase, dtype=np.float32)
    n, d = mag.shape
    n_cores = 8
    nc = _get_program(n, d, n_cores)
    in_maps = make_inputs(mag, phase, n_cores=n_cores)
    res = run_bass_kernel_spmd(nc, in_maps, list(range(n_cores)))
    return gather_outputs(res.results, n, d, n_cores=n_cores)


# revision 34
# speedup vs baseline: 1.4750x; 1.0477x over previous
"""ComplexPolarAttention Trainium2 kernel.

score_ij = sum_d mag_i,d mag_j,d cos(phase_i,d - phase_j,d)
         = a_i . a_j + b_i . b_j         with a = mag*cos(phase), b = mag*sin(phase)
out_mag   = softmax(score, axis=1) @ mag
out_phase = softmax(score, axis=1) @ phase

Strategy (8 NeuronCores, SPMD, no collectives):
  - Rows (queries) sharded; keys replicated. Per-core inputs are ROTATED
    along the key axis so core c's queries are columns 0..q of its own key
    panel (softmax over keys is permutation invariant), so the query
    operand is a slice of the key panel.
  - Scores are computed transposed, S^T[k_blk=128, q=1024] in PSUM via one
    K=128 f32r matmul pair per key block (exact).  exp is NOT normalized
    (scores < 88 cannot overflow) and is written as BF16 -- bf16's 8-bit
    exponent covers the ~15-octave dynamic range of the attention weights
    (fp8 flushes the off-diagonal mass and fails; measured).
  - exp runs split across TWO engines so it stays off the critical path:
    ACT does most blocks (true exp table); DVE does the rest with a
    one-instruction bf16 Schraudolph (uint16 = s*184.665 + 16248.6, the
    f32->uint16 write rounds to nearest; bit pattern IS bf16 exp(s) within
    +-4%, mean-centered).  The tiny per-weight wobble cancels in the
    softmax normalization (verified: end-to-end rel err ~3e-3).
  - Value matmuls use a PACKED [mag|phase] bf16 stationary (M=128, the full
    PE array) -- halving the baseline's value-matmul column count.  The
    softmax denominator that baseline got from a 65th 'ones' column is
    instead accumulated on the otherwise-idle DVE as a bf16 binary tree
    over the es tiles, finished by one cheap f32r ones-matmul
    (cross-partition sum) at the tail.
  - PSUM: 2x score tiles (4 banks) + 2 value accumulators (2 banks) +
    2 denominator tiles (2 banks) = 8 banks exactly.
"""

import numpy as np
import ml_dtypes
from contextlib import ExitStack

import concourse.bass as bass
import concourse.tile as tile
from concourse import bacc, mybir
from concourse.bass_utils import run_bass_kernel_spmd

F32 = mybir.dt.float32
F32R = mybir.dt.float32r
BF16 = mybir.dt.bfloat16
U16 = mybir.dt.uint16
BF = ml_dtypes.bfloat16

A16 = 184.6649652       # 128 / ln 2
B16 = 16248.6           # 127*128 - 7.4 (mean-centering Schraudolph offset)


def abt_chunk_widths(n):
    """First chunk covers the whole query panel (cols 0..1023) so one DMA
    gates the first score matmul; the rest stream in 1024-wide."""
    widths, rem = [], n
    while rem:
        w = min(1024, rem)
        widths.append(w)
        rem -= w
    return widths


def engine_plan(kblocks, n_dve):
    """'A' (ACT exp) / 'D' (DVE schraudolph) per key block, spread evenly.
    The first 8 blocks (diagonal) and the last 2 (tail latency) stay on
    ACT, whose queue drains in lock-step with the PE."""
    plan = ['A'] * kblocks
    if n_dve <= 0:
        return plan
    lo, hi = 8, kblocks - 6
    stride = (hi - lo) / n_dve
    placed = 0
    for i in range(n_dve):
        pos = lo + int(i * stride + stride / 2)
        if plan[pos] == 'A':
            plan[pos] = 'D'
            placed += 1
    for j in range(hi - 1, lo - 1, -1):
        if placed >= n_dve:
            break
        if plan[j] == 'A':
            plan[j] = 'D'
            placed += 1
    return plan


N_DVE_EXP = 14          # blocks of exp done on DVE (rest on ACT)
VLAG = 3                # value matmuls trail the exp stream by 3 blocks
GP_L1_PAIRS = {2, 6, 10, 14, 18, 22, 26}       # den L1 adds on GpSimd


def build_program(n=8192, d=64, n_cores=8, enable_asserts=False):
    assert d == 64
    q = n // n_cores
    kblocks = n // 128
    qblk = 512
    nhalf = q // qblk
    assert nhalf == 2 and q == 1024

    nc = bacc.Bacc(
        "TRN2",
        target_bir_lowering=False,
        debug=False,
        enable_asserts=enable_asserts,
        num_devices=n_cores,
    )

    # ---- DRAM I/O (per-core arrays rotated so queries = keys[0:q]) ----
    chunks = abt_chunk_widths(n)
    abt_in = [nc.dram_tensor(f"abt{i}", [128, w], BF16,
                             kind="ExternalInput").ap()
              for i, w in enumerate(chunks)]
    VCH = 8
    vch = kblocks // VCH
    mopv_in = nc.dram_tensor("mopv", [VCH, 128, vch * 128], BF16,
                             kind="ExternalInput").ap()

    om = nc.dram_tensor("om", [128, q], F32, kind="ExternalOutput").ap()
    # bf16 per-partition denominator partials; host does the final
    # 128-row sum (frees 2 PSUM banks -> 3rd score buffer below)
    oden = nc.dram_tensor("oden", [128, q], BF16, kind="ExternalOutput").ap()

    ENG = engine_plan(kblocks, N_DVE_EXP)

    with tile.TileContext(nc) as tc, ExitStack() as ctx:
        persist = ctx.enter_context(tc.tile_pool(name="persist", bufs=1))
        epool = ctx.enter_context(tc.tile_pool(name="exps", bufs=10))
        tpool = [ctx.enter_context(tc.tile_pool(name=f"tree{i}", bufs=2))
                 for i in range(5)]
        opool = ctx.enter_context(tc.tile_pool(name="outs", bufs=2))
        spool = ctx.enter_context(tc.tile_pool(name="scores", bufs=3,
                                               space="PSUM"))
        vpool = ctx.enter_context(tc.tile_pool(name="vaccum", bufs=1,
                                               space="PSUM"))

        abt = persist.tile([128, n], BF16)            # [a|b]^T all keys
        mopv = persist.tile([128, kblocks, 128], BF16)  # [mag|phase] values

        # key panel chunks on the sync HWDGE queue (chunk 0 gates block 0)
        off = 0
        for i, w in enumerate(chunks):
            nc.sync.dma_start(out=abt[:, off:off + w], in_=abt_in[i])
            off += w
        abq = abt[:, 0:q]

        # value stationaries on the gpsimd SWDGE queue (idle otherwise)
        for vi in range(VCH):
            b0 = vi * vch
            nc.gpsimd.dma_start(out=mopv[:, b0:b0 + vch, :],
                                in_=mopv_in[vi, :, :])

        # PSUM accumulators
        psV = [vpool.tile([128, qblk], F32, name=f"psV{j}", tag=f"psV{j}")
               for j in range(2)]

        # Brief PE warm-up (HAM): measured traces show this device enters
        # the kernel already at full clock, so 2 matmuls are insurance for
        # a cold start only -- more just delays the first real matmul,
        # which is gated by the query-panel DMA (~10us) anyway.
        wsrc = persist.tile([128, 512], F32)
        nc.vector.memset(wsrc[:, :], 0.0)
        warm = spool.tile([128, q], F32, name="warm", tag="ss")
        for _ in range(2):      # fp32 dummies: 4 cyc/row
            nc.tensor.matmul(out=warm[0:16, 0:512], lhsT=wsrc[:, 0:16],
                             rhs=wsrc[:, 0:512], start=True, stop=True)

        # Denominator binary tree, all-bf16 (f32/in-place DVE adds run at
        # 1x and serialize -- measured 1602ns vs 653ns for bf16 ones).
        # L1 tiles (block pairs) enter at level 1; 32 -> 16 -> 8 -> 4 ->
        # 2 -> 1.  A quarter of the L1 adds run on the otherwise-idle
        # GpSimd (~2.8us each there, but off the DVE critical engine).
        pend = [None] * 7
        acc_holder = [None]

        def tree_feed(level, t):
            if level == 6:
                acc_holder[0] = t
                return
            if pend[level] is None:
                pend[level] = t
                return
            s = pend[level]
            pend[level] = None
            if level < 5:
                o = tpool[min(level, 4)].tile([128, q], BF16,
                                              name=f"t{level}")
            else:
                o = persist.tile([128, q], BF16, name="denacc")
            nc.vector.tensor_tensor(out=o[:, :], in0=s[:, :], in1=t[:, :],
                                    op=mybir.AluOpType.add)
            tree_feed(level + 1, o)

        def value_mms(es, kb, first, last):
            for j in range(2):
                nc.tensor.matmul(
                    out=psV[j][:, :],
                    lhsT=mopv[:, kb, :],
                    rhs=es[:, j * qblk:(j + 1) * qblk],
                    start=first, stop=last,
                )

        es_hist = []
        for kb in range(kblocks):
            ss = spool.tile([128, q], F32)
            for j in range(2):
                nc.tensor.matmul(
                    out=ss[:, j * qblk:(j + 1) * qblk],
                    lhsT=abt[:, kb * 128:(kb + 1) * 128],
                    rhs=abq[:, j * qblk:(j + 1) * qblk],
                    start=True, stop=True,
                )
            es = epool.tile([128, q], BF16)
            if ENG[kb] == 'A':
                nc.scalar.activation(
                    es[:, :], ss[:, :], mybir.ActivationFunctionType.Exp,
                )
            else:
                nc.vector.tensor_scalar(
                    out=es[:, :].bitcast(U16), in0=ss[:, :],
                    scalar1=A16, scalar2=B16,
                    op0=mybir.AluOpType.mult, op1=mybir.AluOpType.add,
                )
            es_hist.append(es)
            # den tree leaf: pair the two newest es tiles
            if kb % 2 == 1:
                pair = kb // 2
                eng = nc.gpsimd if pair in GP_L1_PAIRS else nc.vector
                l1 = tpool[0].tile([128, q], BF16)
                eng.tensor_tensor(out=l1[:, :], in0=es_hist[-2][:, :],
                                  in1=es_hist[-1][:, :],
                                  op=mybir.AluOpType.add)
                tree_feed(1, l1)
            if len(es_hist) > VLAG:
                value_mms(es_hist[-1 - VLAG], kb - VLAG,
                          first=(kb == VLAG), last=False)
        for i in range(VLAG, 0, -1):
            value_mms(es_hist[-i], kblocks - i,
                      first=False, last=(i == 1))

        # denominator partials straight to DRAM (host sums the 128 rows)
        denacc = acc_holder[0]
        assert denacc is not None
        nc.sync.dma_start(out=oden, in_=denacc[:, :])

        # outputs: PSUM -> SBUF (DVE) -> DRAM
        oV = opool.tile([128, q], F32, tag="oV")
        for j in range(2):
            qsl = slice(j * qblk, (j + 1) * qblk)
            nc.vector.tensor_copy(oV[:, qsl], psV[j][:, :])
        nc.sync.dma_start(out=om, in_=oV[:, :])

    nc.compile()
    return nc


def make_inputs(mag, phase, n_cores=8):
    """Host-side prep -> per-core (key-rotated) input maps."""
    n, d = mag.shape
    q = n // n_cores
    kblocks = n // 128
    mag = np.ascontiguousarray(mag, dtype=np.float32)
    phase = np.ascontiguousarray(phase, dtype=np.float32)

    a = mag * np.cos(phase)
    b = mag * np.sin(phase)
    abt_g = np.concatenate([a.T, b.T], axis=0).astype(BF)          # [128, n]
    V = np.concatenate([mag, phase], axis=1).astype(BF)            # [n, 128]

    chunks = abt_chunk_widths(n)
    VCH = 8
    vch = kblocks // VCH

    in_maps = []
    for c in range(n_cores):
        r = c * q
        abt_c = np.roll(abt_g, -r, axis=1)
        Vr = np.roll(V, -r, axis=0)
        # mopv[vi][p][blk*128 + f] = V[(vi*vch+blk)*128 + p, f]
        mo = np.ascontiguousarray(
            Vr.reshape(VCH, vch, 128, 128).transpose(0, 2, 1, 3)
            .reshape(VCH, 128, vch * 128))
        m = {"mopv": mo}
        off = 0
        for i, w in enumerate(chunks):
            m[f"abt{i}"] = np.ascontiguousarray(abt_c[:, off:off + w])
            off += w
        in_maps.append(m)
    return in_maps


def gather_outputs(results, n, d, n_cores=8):
    new_mag = np.empty((n, d), np.float32)
    new_phase = np.empty((n, d), np.float32)
    q = n // n_cores
    for c in range(n_cores):
        om = results[c]["om"]            # [128, q]: rows 0:64 mag, 64:128 ph
        den = results[c]["oden"].astype(np.float32).sum(axis=0)   # [q]
        qsl = slice(c * q, (c + 1) * q)
        new_mag[qsl] = (om[0:64, :] / den).T
        new_phase[qsl] = (om[64:128, :] / den).T
    return new_mag, new_phase


_PROGRAM_CACHE = {}


def _get_program(n, d, n_cores):
    key = (n, d, n_cores)
    if key not in _PROGRAM_CACHE:
        _PROGRAM_CACHE[key] = build_program(n=n, d=d, n_cores=n_cores)
    return _PROGRAM_CACHE[key]


def kernel(mag, phase):
    mag = np.asarray(mag, dtype=np.float32)
    phase = np.asarray(phase, dtype=np.float32)
    n, d = mag.shape
    n_cores = 8
    nc = _get_program(n, d, n_cores)
    in_maps = make_inputs(mag, phase, n_cores=n_cores)
    res = run_bass_kernel_spmd(nc, in_maps, list(range(n_cores)))
    return gather_outputs(res.results, n, d, n_cores=n_cores)
